# revision 28
# baseline (speedup 1.0000x reference)
"""MetaOptNet episode kernel for 8x Trainium2 NeuronCores.

Math (from the reference nn.Module):
    x: [15025, 4096] = 5 classes x (5 support + 3000 query) rows.
    K = support @ support.T  (25x25)
    qp = interior-point solve of a tiny 125-var SVM dual (15 fixed iterations)
    logits = (query @ support.T) @ qp        -> [15000, 5]

Split of work:
  - The QP solve is a tiny serial 125-variable problem; it is replicated on
    the host in float32, exactly mirroring the reference algorithm.
  - The memory-bound bulk (streaming the 245 MB of query rows against
    W = support.T @ qp) runs on the 8 NeuronCores, data-parallel over query
    rows (1875 queries per core).

Device kernel design (per core):
  - The query stream is quantized host-side to fp8 e3m4 (x * 2, exactly
    invertible scale), quartering HBM traffic vs the fp32 baseline. W is
    carried as an e3m4 hi+lo pair (W * 2048), making its quantization error
    negligible; the logits are divided by 4096 on the host afterwards.
    End-to-end relative error ~1.1e-2 (gate: 2e-2), dominated by the x
    quantization, verified deterministically on the fixed episode inputs.
  - Matmuls run x-stationary: lhsT = a [128, 125] feature-major query tile
    (PE stationary array), rhs = the [128, 8] W chunk (moving). Each query
    tile accumulates its 32 k-chunks x 2 planes into a private PSUM bank
    ([125, 8] f32); accumulation groups must not share a PSUM bank, so at
    most 8 tiles are in flight and tiles cycle through 6 banks.
  - Queries are streamed chunk-major (chunks of 3/3/3/3/2/1 tiles of 125).
    A chunk's k-slabs arrive as [128, kslab, csz] fp8 DMAs (>=2KB per
    partition row, full 360 GB/s); its tiles' outputs are copied and stored
    while later chunks stream, so only the tiny last chunk sits in the
    serial tail.
"""

import os

import numpy as np

# ---------------------------------------------------------------- constants
N_WAY = 5
N_SUPPORT = 5
N_QUERY = 3000
D = 4096
C_REG = 0.1
MAX_ITER = 15
SIGMA = 0.1

N_CORES = 8
NS = N_WAY * N_SUPPORT          # 25 support rows
NQ_TOT = N_WAY * N_QUERY        # 15000 query rows
NQ_SHARD = NQ_TOT // N_CORES    # 1875 per core
KCH = D // 128                  # 32 contraction chunks of 128
TSZ = 125                       # query tile rows (PSUM group partition dim)
NT_TOT = NQ_SHARD // TSZ        # 15 tiles per core
NW_PAD = 8                      # classes padded (zero) to 8

SX = 2.0                        # x quantization scale (power of 2, exact)
SW_TARGET = 15.0                # per-column W scale target absmax (e3m4 max 15.5)

# chunk layout: "tiles:kslab+kslab+...," per chunk (tiles sum to NT_TOT);
# small chunk last (with a tiny final k-slab) so the serial tail after the
# stream is minimal
_CHUNK_SPEC = os.environ.get(
    "MK_CHUNKS", "3:32,3:32,3:32,3:32,2:32,1:24+8"
)
CHUNKS = tuple(
    (int(part.split(":")[0]),
     tuple(int(s) for s in part.split(":")[1].split("+")))
    for part in _CHUNK_SPEC.split(",")
)
CHUNK_TILES = tuple(nt for nt, _ in CHUNKS)
N_BANKS = int(os.environ.get("MK_BANKS", "6"))  # PSUM banks cycled by tiles
# experimental: last chunk's logits via a pre-generated SWDGE scatter
# descriptor fired by a trigger, skipping the HWDGE+DGE setup (~0.7us) in
# the tail. Default OFF: the prep/trigger path miscompiles in neuronxcc
# (setupSyncUpdate codegen crash) and its completion semaphore bookkeeping
# deadlocks the TimelineSim end barrier without manual sync surgery.
USE_TRIG = os.environ.get("MK_TRIG", "0") == "1"
assert sum(CHUNK_TILES) == NT_TOT
assert all(sum(ks) == KCH for _, ks in CHUNKS)
assert not USE_TRIG or CHUNK_TILES[-1] == 1


def _chunk_starts():
    starts, t0 = [], 0
    for nt in CHUNK_TILES:
        starts.append(t0)
        t0 += nt
    return starts


# ------------------------------------------------------------ host QP solve
def _qp_solve_host(K):
    """Mirror of reference._qp_solve for this problem's fixed G/e/C/h/A/b.

    C is the identity and b is zero, so C-products are elided (exact in
    fp32).  All arithmetic in float32 to track the reference's rounding.
    """
    dt = np.float32
    n = NS * N_WAY                                    # 125
    m, p = n, NS                                      # 125, 25
    G = np.kron(K, np.eye(N_WAY, dtype=dt)).astype(dt) + np.eye(n, dtype=dt)
    y = np.repeat(np.arange(N_WAY), N_SUPPORT)
    y1 = np.eye(N_WAY, dtype=dt)[y].reshape(-1)       # [125] one-hot flat
    e = -y1
    h = (dt(C_REG) * y1).astype(dt)
    A = np.kron(np.eye(NS, dtype=dt), np.ones((1, N_WAY), dtype=dt)).astype(dt)
    sigma = dt(SIGMA)

    z = np.zeros(n, dt)
    s = np.ones(m, dt)
    lam = np.ones(m, dt)
    nu = np.zeros(p, dt)

    for _ in range(MAX_ITER):
        r_dual = G @ z + e + lam + A.T @ nu
        r_pin = z + s - h
        r_peq = A @ z
        mu = np.dot(s, lam) / dt(m)
        r_cent = s * lam - sigma * mu
        w = lam / s
        M = G + np.diag(w).astype(dt)
        rhs_z = -(r_dual + (-r_cent + lam * r_pin) / s)
        KKT = np.block([[M, A.T], [A, np.zeros((p, p), dt)]]).astype(dt)
        sol = np.linalg.solve(KKT, np.concatenate([rhs_z, -r_peq]))
        dz, dnu = sol[:n], sol[n:]
        ds = -r_pin - dz
        dlam = (-r_cent - lam * ds) / s
        with np.errstate(divide="ignore", invalid="ignore"):
            a_s = np.min(np.where(ds < 0, -s / ds, np.inf)).astype(dt)
            a_l = np.min(np.where(dlam < 0, -lam / dlam, np.inf)).astype(dt)
        alpha = np.minimum(dt(1.0), dt(0.99) * np.minimum(a_s, a_l))
        z = z + alpha * dz
        s = s + alpha * ds
        lam = lam + alpha * dlam
        nu = nu + alpha * dnu

    return z.reshape(NS, N_WAY)                       # [25, 5]


# ------------------------------------------------------------- bass builder
_BUILD_CACHE = {}


def _np_f8():
    import ml_dtypes

    return np.dtype(ml_dtypes.float8_e3m4)


def _build_bass():
    key = (CHUNKS, N_BANKS, USE_TRIG)
    if key in _BUILD_CACHE:
        return _BUILD_CACHE[key]

    import concourse.bacc as bacc
    import concourse.mybir as mybir
    import concourse.tile as tile

    f8 = mybir.dt.float8e3
    f32 = mybir.dt.float32
    i16 = mybir.dt.int16

    nc = bacc.Bacc(
        "TRN2", target_bir_lowering=False, debug=False, num_swdge_queues=2
    )
    # chunk 0 carries W appended per k-chunk: [csz stream | 2*NW_PAD W bytes]
    # so no separate W DMA is needed (8-HWDGE-semaphore budget: reusing a
    # semaphore makes a later DMA wait on an unrelated earlier one)
    xts = [
        nc.dram_tensor(
            f"xt{g}",
            [128, KCH, nt * TSZ + (2 * NW_PAD if g == 0 else 0)],
            f8,
            kind="ExternalInput",
        )
        for g, nt in enumerate(CHUNK_TILES)
    ]
    outT = nc.dram_tensor("outT", [TSZ, NT_TOT, NW_PAD], f32, kind="ExternalOutput")
    if USE_TRIG:
        # last tile's rows, one 256B-strided slot per query row (scatter-add
        # destination stride must be a multiple of 256 bytes)
        outS = nc.dram_tensor("outS", [128, 64], f32, kind="ExternalOutput")

    starts = _chunk_starts()

    with tile.TileContext(nc) as tc:
        with (
            tc.tile_pool(name="const", bufs=1) as cpool,
            tc.tile_pool(name="stream", bufs=1) as spool,
            tc.tile_pool(name="acc", bufs=1, space="PSUM") as apool,
        ):
            slabs = {}
            for g, (nt, kslabs) in enumerate(CHUNKS):
                csz = nt * TSZ + (2 * NW_PAD if g == 0 else 0)
                k0 = 0
                for ks in kslabs:
                    slab = spool.tile(
                        [128, ks, csz], f8,
                        tag=f"slab{g}_{k0}", name=f"slab{g}_{k0}",
                    )
                    nc.sync.dma_start(slab[:], xts[g][:, k0 : k0 + ks, :])
                    slabs[g, k0] = (slab, ks)
                    k0 += ks

            if USE_TRIG:
                # pre-generate the last tile's scatter-add descriptors while
                # the stream runs; only the trigger sits in the serial tail
                z8 = cpool.tile([128, 8], f32, tag="z8")
                nc.gpsimd.memset(z8[:], 0.0)
                s3 = cpool.tile([128, 1, NW_PAD], f32, tag="s3")
                nc.gpsimd.memset(s3[:], 0.0)
                idxs = cpool.tile([16, 8], i16, tag="idx")
                nc.gpsimd.iota(
                    idxs[:], pattern=[[16, 8]], base=0, channel_multiplier=1
                )
                # scatter-add needs its destination payload region zeroed
                # (Pool queue: no waits, runs early, keeps HWDGE slots free)
                nc.gpsimd.dma_start(outS[:, 0:NW_PAD], z8[:])
                sem_out5 = nc.alloc_semaphore("out5_dma")
                nc.gpsimd.dma_scatter_add(
                    outS[:, 0:NW_PAD],
                    s3[:],
                    idxs[:],
                    128,
                    128,
                    NW_PAD,
                    elem_step=64,
                    prepare_only=True,
                    sem=sem_out5,
                    queue_num=1,
                )
            # W slices live inside chunk0's slabs: w_at(k) -> [128, 2*NW_PAD]
            c0 = CHUNK_TILES[0] * TSZ
            k0s_0 = []
            k0 = 0
            for ks in CHUNKS[0][1]:
                k0s_0.append((k0, ks))
                k0 += ks

            def w_at(k, pl):
                for kk0, ks in k0s_0:
                    if kk0 <= k < kk0 + ks:
                        return slabs[0, kk0][0][
                            :, k - kk0, c0 + pl * NW_PAD : c0 + (pl + 1) * NW_PAD
                        ]
                raise AssertionError(k)

            # one PSUM bank per in-flight query tile; tile i -> bank i % N_BANKS
            accs = [
                apool.tile([128, NW_PAD], f32, tag=f"acc{s}", name=f"acc{s}")
                for s in range(N_BANKS)
            ]
            out_sb = cpool.tile([128, NT_TOT, NW_PAD], f32, tag="out")

            for g, (nt, kslabs) in enumerate(CHUNKS):
                t0 = starts[g]
                k0 = 0
                for ks in kslabs:
                    slab, _ = slabs[g, k0]
                    for tl in range(nt):
                        acc = accs[(t0 + tl) % N_BANKS]
                        for kk in range(ks):
                            k = k0 + kk
                            for pl in range(2):
                                nc.tensor.matmul(
                                    acc[:TSZ, :],
                                    slab[:, kk, tl * TSZ : (tl + 1) * TSZ],
                                    w_at(k, pl),
                                    start=(k == 0 and pl == 0),
                                    stop=(k == KCH - 1 and pl == 1),
                                )
                    k0 += ks
                # chunk done: drain its PSUM banks and store its logits.
                # Early outs ride the Pool SWDGE queue (own semaphore space,
                # desc-gen on the otherwise idle Pool engine) so the stream's
                # HWDGE semaphores are never entangled with out completions;
                # the final out fires the pre-generated scatter descriptors
                # (or, without MK_TRIG, a plain SP DMA).
                last = g == len(CHUNKS) - 1
                if last and USE_TRIG:
                    nc.vector.tensor_copy(
                        s3[:TSZ, 0, :], accs[(t0 + 0) % N_BANKS][:TSZ, :]
                    )
                    nc.gpsimd.trigger_dma(count=None, queue_num=1)
                    # completion wait on SP: Pool's sequencer must stay free
                    # for the trigger's descriptor-replay track to run
                    nc.sync.wait_ge(sem_out5, 16)
                else:
                    for tl in range(nt):
                        nc.vector.tensor_copy(
                            out_sb[:TSZ, t0 + tl, :],
                            accs[(t0 + tl) % N_BANKS][:TSZ, :],
                        )
                    # next-to-last chunk's out must not sit on Pool's
                    # sequencer ahead of the trigger; SP is free by then
                    late = USE_TRIG and g == len(CHUNKS) - 2
                    out_eng = nc.sync if (last or late) else nc.gpsimd
                    out_eng.dma_start(
                        outT[:, t0 : t0 + nt, :], out_sb[:TSZ, t0 : t0 + nt, :]
                    )

    if USE_TRIG:
        # The tile sem-assignment schedules the scatter prep on a rotating
        # DMASW lane but the descriptor's completion rides our explicit
        # out5_dma semaphore, so the end-barrier's DMASW wait dangles with
        # no incrementer (deadlock). Drop danglers: program-end ordering is
        # still enforced by the explicit wait_ge(out5_dma) on Pool.
        def _walk(blocks):
            for b in blocks:
                for inst in b.instructions:
                    yield inst
                    if getattr(inst, "blocks", None):
                        yield from _walk(inst.blocks)

        updated = set()
        insts = list(_walk(nc.m.functions[0].blocks))
        for inst in insts:
            si = inst.sync_info
            if si:
                for u in si.on_update:
                    updated.add(u.ant_name)
        for inst in insts:
            si = inst.sync_info
            if si and si.on_wait:
                keep = [
                    w for w in si.on_wait
                    if not (
                        (w.ant_name or "").startswith("DMASW")
                        and w.ant_name not in updated
                    )
                ]
                if len(keep) != len(si.on_wait):
                    si.on_wait = keep

    nc.compile()
    _BUILD_CACHE[key] = nc
    return nc


# ------------------------------------------------------------ input packing
def _pack_shards(query, whl):
    """query [15000, 4096] f32 -> per-core dict of chunk tensors.

    whl [128, KCH, 2, NW_PAD] e3m4 W planes are appended to chunk 0's
    per-k columns so the whole episode needs no separate W DMA.
    """
    f8 = _np_f8()
    xq = (query * np.float32(SX)).astype(f8)          # [15000, 4096] e3m4
    wcols = whl.reshape(128, KCH, 2 * NW_PAD)
    starts = _chunk_starts()
    shards = []
    for c in range(N_CORES):
        qs = xq[c * NQ_SHARD : (c + 1) * NQ_SHARD]    # [1875, 4096]
        chunk_map = {}
        for g, nt in enumerate(CHUNK_TILES):
            csz = nt * TSZ
            q0 = starts[g] * TSZ
            blk = qs[q0 : q0 + csz]                   # [csz, 4096]
            # [csz, KCH, 128] -> [128, KCH, csz]
            arr = blk.reshape(csz, KCH, 128).transpose(2, 1, 0)
            if g == 0:
                arr = np.concatenate([arr, wcols], axis=2)
            chunk_map[f"xt{g}"] = np.ascontiguousarray(arr)
        shards.append(chunk_map)
    return shards


def _pack_w(support, qp):
    """W = sup.T @ qp [4096, 5] -> e3m4 hi/lo planes [128, KCH, 2, NW_PAD].

    Each class column gets its own power-of-2 scale pushing it to the top of
    e3m4's normal range, so the lo plane's residual (denormal floor) is as
    small as possible relative to the column. Returns (whl, col_scales).
    """
    f8 = _np_f8()
    W = np.zeros((D, NW_PAD), np.float32)
    W[:, :N_WAY] = support.T @ qp
    absmax = np.abs(W).max(axis=0)
    scales = np.where(
        absmax > 0,
        np.exp2(np.floor(np.log2(SW_TARGET / np.maximum(absmax, 1e-30)))),
        1.0,
    ).astype(np.float32)
    Wt = W * scales[None, :]
    whi = Wt.astype(f8)
    wlo = (Wt - whi.astype(np.float32)).astype(f8)
    whl = np.zeros((128, KCH, 2, NW_PAD), f8)
    for pl, w in enumerate((whi, wlo)):
        whl[:, :, pl, :] = w.reshape(KCH, 128, NW_PAD).transpose(1, 0, 2)
    return np.ascontiguousarray(whl), scales


def kernel(x):
    x = np.ascontiguousarray(np.asarray(x, dtype=np.float32))
    xr = x.reshape(N_WAY, N_SUPPORT + N_QUERY, D)
    support = np.ascontiguousarray(xr[:, :N_SUPPORT].reshape(NS, D))
    query = np.ascontiguousarray(xr[:, N_SUPPORT:].reshape(NQ_TOT, D))

    # --- host: tiny QP solve (replicated, mirrors reference numerics)
    K = support @ support.T
    qp = _qp_solve_host(K)                              # [25, 5] f32

    whl, col_scales = _pack_w(support, qp)
    shards = _pack_shards(query, whl)

    in_maps = [dict(shards[c]) for c in range(N_CORES)]

    res = None
    last_err = None
    for attempt in range(3):
        try:
            from concourse.bass_utils import run_bass_kernel_spmd

            nc = _build_bass()
            res = run_bass_kernel_spmd(
                nc, in_maps, core_ids=list(range(N_CORES))
            )
            break
        except Exception as e:  # transient device/compile hiccups
            last_err = e
            import sys, time, traceback

            traceback.print_exc()
            word = "retrying" if attempt < 2 else "giving up"
            print(
                f"kernel: device attempt {attempt} failed "
                f"({type(e).__name__}), {word}",
                file=sys.stderr,
            )
            time.sleep(2.0 * (attempt + 1))

    inv = (1.0 / (SX * col_scales[:N_WAY])).astype(np.float32)
    if res is not None:
        logits = np.empty((NQ_TOT, N_WAY), np.float32)
        for c in range(N_CORES):
            outT = np.array(res.results[c]["outT"])     # [125, 15, 8]
            if USE_TRIG:
                outT[:, NT_TOT - 1, :] = res.results[c]["outS"][:TSZ, :NW_PAD]
            logits[c * NQ_SHARD : (c + 1) * NQ_SHARD] = (
                outT.transpose(1, 0, 2).reshape(NQ_SHARD, NW_PAD)[:, :N_WAY]
                * inv[None, :]
            )
        return logits

    # last-resort host fallback: numerically correct, no device speedup
    import sys

    print(
        f"kernel: falling back to host compute after device failure: "
        f"{last_err!r}",
        file=sys.stderr,
    )
    return ((query @ support.T) @ qp).astype(np.float32)


# revision 30
# speedup vs baseline: 1.0006x; 1.0006x over previous
"""MetaOptNet episode kernel for 8x Trainium2 NeuronCores.

Math (from the reference nn.Module):
    x: [15025, 4096] = 5 classes x (5 support + 3000 query) rows.
    K = support @ support.T  (25x25)
    qp = interior-point solve of a tiny 125-var SVM dual (15 fixed iterations)
    logits = (query @ support.T) @ qp        -> [15000, 5]

Split of work:
  - The QP solve is a tiny serial 125-variable problem; it is replicated on
    the host in float32, exactly mirroring the reference algorithm.
  - The memory-bound bulk (streaming the 245 MB of query rows against
    W = support.T @ qp) runs on the 8 NeuronCores, data-parallel over query
    rows (1875 queries per core).

Device kernel design (per core):
  - The query stream is quantized host-side to fp8 e3m4 (x * 2, exactly
    invertible scale), quartering HBM traffic vs the fp32 baseline. W is
    carried as an e3m4 hi+lo pair with per-class power-of-2 scales pushing
    each column to the top of e3m4's normal range, making its quantization
    error negligible; the host divides each logit column by its scale
    afterwards. End-to-end relative error ~1.34e-2 (gate: 2e-2), dominated
    by the x quantization, deterministic on the fixed episode inputs.
  - Matmuls run x-stationary: lhsT = a [128, 125] feature-major query tile
    (PE stationary array, whose load the PE pipelines behind the moving
    pass), rhs = the [128, 8] W chunk (moving, out free size 8) — so the
    tensor engine is far off the critical path. Each query tile accumulates
    its 32 k-chunks x 2 W planes into a private PSUM bank ([125, 8] f32);
    accumulation groups must not share a PSUM bank (bank-granular
    accumulate), so tiles cycle through 6 banks.
  - Queries stream chunk-major (chunks of 3/3/3/3/2/1 tiles of 125): a
    chunk's k-slabs arrive as [128, kslab, csz] fp8 DMAs (>=2KB/partition
    row, full 360 GB/s); outputs are copied and stored while later chunks
    stream, so only the tiny last chunk (+ its 12 post-stream matmuls)
    sits in the serial tail.
  - DMA instruction budget is tuned to the 8 HWDGE + 8 SWDGE completion
    semaphores: 7 stream DMAs + the last out on SP/Activation HWDGE (W
    rides inside chunk 0's stream tensor), the other outs on the Pool
    SWDGE queue — semaphore-slot reuse would chain unrelated DMAs.
"""

import os

import numpy as np

# ---------------------------------------------------------------- constants
N_WAY = 5
N_SUPPORT = 5
N_QUERY = 3000
D = 4096
C_REG = 0.1
MAX_ITER = 15
SIGMA = 0.1

N_CORES = 8
NS = N_WAY * N_SUPPORT          # 25 support rows
NQ_TOT = N_WAY * N_QUERY        # 15000 query rows
NQ_SHARD = NQ_TOT // N_CORES    # 1875 per core
KCH = D // 128                  # 32 contraction chunks of 128
TSZ = 125                       # query tile rows (PSUM group partition dim)
NT_TOT = NQ_SHARD // TSZ        # 15 tiles per core
NW_PAD = 8                      # classes padded (zero) to 8

SX = 2.0                        # x quantization scale (power of 2, exact)
SW_TARGET = 15.0                # per-column W scale target absmax (e3m4 max 15.5)

# chunk layout: "tiles:kslab+kslab+...," per chunk (tiles sum to NT_TOT);
# small chunk last (with a tiny final k-slab) so the serial tail after the
# stream is minimal
_CHUNK_SPEC = os.environ.get(
    "MK_CHUNKS", "3:32,3:32,3:32,3:32,2:32,1:26+6"
)
CHUNKS = tuple(
    (int(part.split(":")[0]),
     tuple(int(s) for s in part.split(":")[1].split("+")))
    for part in _CHUNK_SPEC.split(",")
)
CHUNK_TILES = tuple(nt for nt, _ in CHUNKS)
N_BANKS = int(os.environ.get("MK_BANKS", "6"))  # PSUM banks cycled by tiles
# experimental: last chunk's logits via a pre-generated SWDGE scatter
# descriptor fired by a trigger, skipping the HWDGE+DGE setup (~0.7us) in
# the tail. Default OFF: the prep/trigger path miscompiles in neuronxcc
# (setupSyncUpdate codegen crash) and its completion semaphore bookkeeping
# deadlocks the TimelineSim end barrier without manual sync surgery.
USE_TRIG = os.environ.get("MK_TRIG", "0") == "1"
assert sum(CHUNK_TILES) == NT_TOT
assert all(sum(ks) == KCH for _, ks in CHUNKS)
assert not USE_TRIG or CHUNK_TILES[-1] == 1


def _chunk_starts():
    starts, t0 = [], 0
    for nt in CHUNK_TILES:
        starts.append(t0)
        t0 += nt
    return starts


# ------------------------------------------------------------ host QP solve
def _qp_solve_host(K):
    """Mirror of reference._qp_solve for this problem's fixed G/e/C/h/A/b.

    C is the identity and b is zero, so C-products are elided (exact in
    fp32).  All arithmetic in float32 to track the reference's rounding.
    """
    dt = np.float32
    n = NS * N_WAY                                    # 125
    m, p = n, NS                                      # 125, 25
    G = np.kron(K, np.eye(N_WAY, dtype=dt)).astype(dt) + np.eye(n, dtype=dt)
    y = np.repeat(np.arange(N_WAY), N_SUPPORT)
    y1 = np.eye(N_WAY, dtype=dt)[y].reshape(-1)       # [125] one-hot flat
    e = -y1
    h = (dt(C_REG) * y1).astype(dt)
    A = np.kron(np.eye(NS, dtype=dt), np.ones((1, N_WAY), dtype=dt)).astype(dt)
    sigma = dt(SIGMA)

    z = np.zeros(n, dt)
    s = np.ones(m, dt)
    lam = np.ones(m, dt)
    nu = np.zeros(p, dt)

    for _ in range(MAX_ITER):
        r_dual = G @ z + e + lam + A.T @ nu
        r_pin = z + s - h
        r_peq = A @ z
        mu = np.dot(s, lam) / dt(m)
        r_cent = s * lam - sigma * mu
        w = lam / s
        M = G + np.diag(w).astype(dt)
        rhs_z = -(r_dual + (-r_cent + lam * r_pin) / s)
        KKT = np.block([[M, A.T], [A, np.zeros((p, p), dt)]]).astype(dt)
        sol = np.linalg.solve(KKT, np.concatenate([rhs_z, -r_peq]))
        dz, dnu = sol[:n], sol[n:]
        ds = -r_pin - dz
        dlam = (-r_cent - lam * ds) / s
        with np.errstate(divide="ignore", invalid="ignore"):
            a_s = np.min(np.where(ds < 0, -s / ds, np.inf)).astype(dt)
            a_l = np.min(np.where(dlam < 0, -lam / dlam, np.inf)).astype(dt)
        alpha = np.minimum(dt(1.0), dt(0.99) * np.minimum(a_s, a_l))
        z = z + alpha * dz
        s = s + alpha * ds
        lam = lam + alpha * dlam
        nu = nu + alpha * dnu

    return z.reshape(NS, N_WAY)                       # [25, 5]


# ------------------------------------------------------------- bass builder
_BUILD_CACHE = {}


def _np_f8():
    import ml_dtypes

    return np.dtype(ml_dtypes.float8_e3m4)


def _build_bass():
    key = (CHUNKS, N_BANKS, USE_TRIG)
    if key in _BUILD_CACHE:
        return _BUILD_CACHE[key]

    import concourse.bacc as bacc
    import concourse.mybir as mybir
    import concourse.tile as tile

    f8 = mybir.dt.float8e3
    f32 = mybir.dt.float32
    i16 = mybir.dt.int16

    nc = bacc.Bacc(
        "TRN2", target_bir_lowering=False, debug=False, num_swdge_queues=2
    )
    # chunk 0 carries W appended per k-chunk: [csz stream | 2*NW_PAD W bytes]
    # so no separate W DMA is needed (8-HWDGE-semaphore budget: reusing a
    # semaphore makes a later DMA wait on an unrelated earlier one)
    xts = [
        nc.dram_tensor(
            f"xt{g}",
            [128, KCH, nt * TSZ + (2 * NW_PAD if g == 0 else 0)],
            f8,
            kind="ExternalInput",
        )
        for g, nt in enumerate(CHUNK_TILES)
    ]
    outT = nc.dram_tensor("outT", [TSZ, NT_TOT, NW_PAD], f32, kind="ExternalOutput")
    if USE_TRIG:
        # last tile's rows, one 256B-strided slot per query row (scatter-add
        # destination stride must be a multiple of 256 bytes)
        outS = nc.dram_tensor("outS", [128, 64], f32, kind="ExternalOutput")

    starts = _chunk_starts()

    with tile.TileContext(nc) as tc:
        with (
            tc.tile_pool(name="const", bufs=1) as cpool,
            tc.tile_pool(name="stream", bufs=1) as spool,
            tc.tile_pool(name="acc", bufs=1, space="PSUM") as apool,
        ):
            slabs = {}
            for g, (nt, kslabs) in enumerate(CHUNKS):
                csz = nt * TSZ + (2 * NW_PAD if g == 0 else 0)
                k0 = 0
                for ks in kslabs:
                    slab = spool.tile(
                        [128, ks, csz], f8,
                        tag=f"slab{g}_{k0}", name=f"slab{g}_{k0}",
                    )
                    nc.sync.dma_start(slab[:], xts[g][:, k0 : k0 + ks, :])
                    slabs[g, k0] = (slab, ks)
                    k0 += ks

            if USE_TRIG:
                # pre-generate the last tile's scatter-add descriptors while
                # the stream runs; only the trigger sits in the serial tail
                z8 = cpool.tile([128, 8], f32, tag="z8")
                nc.gpsimd.memset(z8[:], 0.0)
                s3 = cpool.tile([128, 1, NW_PAD], f32, tag="s3")
                nc.gpsimd.memset(s3[:], 0.0)
                idxs = cpool.tile([16, 8], i16, tag="idx")
                nc.gpsimd.iota(
                    idxs[:], pattern=[[16, 8]], base=0, channel_multiplier=1
                )
                # scatter-add needs its destination payload region zeroed
                # (Pool queue: no waits, runs early, keeps HWDGE slots free)
                nc.gpsimd.dma_start(outS[:, 0:NW_PAD], z8[:])
                sem_out5 = nc.alloc_semaphore("out5_dma")
                nc.gpsimd.dma_scatter_add(
                    outS[:, 0:NW_PAD],
                    s3[:],
                    idxs[:],
                    128,
                    128,
                    NW_PAD,
                    elem_step=64,
                    prepare_only=True,
                    sem=sem_out5,
                    queue_num=1,
                )
            # W slices live inside chunk0's slabs: w_at(k) -> [128, 2*NW_PAD]
            c0 = CHUNK_TILES[0] * TSZ
            k0s_0 = []
            k0 = 0
            for ks in CHUNKS[0][1]:
                k0s_0.append((k0, ks))
                k0 += ks

            def w_at(k, pl):
                for kk0, ks in k0s_0:
                    if kk0 <= k < kk0 + ks:
                        return slabs[0, kk0][0][
                            :, k - kk0, c0 + pl * NW_PAD : c0 + (pl + 1) * NW_PAD
                        ]
                raise AssertionError(k)

            # one PSUM bank per in-flight query tile; tile i -> bank i % N_BANKS
            accs = [
                apool.tile([128, NW_PAD], f32, tag=f"acc{s}", name=f"acc{s}")
                for s in range(N_BANKS)
            ]
            out_sb = cpool.tile([128, NT_TOT, NW_PAD], f32, tag="out")

            for g, (nt, kslabs) in enumerate(CHUNKS):
                t0 = starts[g]
                k0 = 0
                for ks in kslabs:
                    slab, _ = slabs[g, k0]
                    for tl in range(nt):
                        acc = accs[(t0 + tl) % N_BANKS]
                        for kk in range(ks):
                            k = k0 + kk
                            for pl in range(2):
                                nc.tensor.matmul(
                                    acc[:TSZ, :],
                                    slab[:, kk, tl * TSZ : (tl + 1) * TSZ],
                                    w_at(k, pl),
                                    start=(k == 0 and pl == 0),
                                    stop=(k == KCH - 1 and pl == 1),
                                )
                    k0 += ks
                # chunk done: drain its PSUM banks and store its logits.
                # Early outs ride the Pool SWDGE queue (own semaphore space,
                # desc-gen on the otherwise idle Pool engine) so the stream's
                # HWDGE semaphores are never entangled with out completions;
                # the final out fires the pre-generated scatter descriptors
                # (or, without MK_TRIG, a plain SP DMA).
                last = g == len(CHUNKS) - 1
                if last and USE_TRIG:
                    nc.vector.tensor_copy(
                        s3[:TSZ, 0, :], accs[(t0 + 0) % N_BANKS][:TSZ, :]
                    )
                    nc.gpsimd.trigger_dma(count=None, queue_num=1)
                    # completion wait on SP: Pool's sequencer must stay free
                    # for the trigger's descriptor-replay track to run
                    nc.sync.wait_ge(sem_out5, 16)
                else:
                    for tl in range(nt):
                        nc.vector.tensor_copy(
                            out_sb[:TSZ, t0 + tl, :],
                            accs[(t0 + tl) % N_BANKS][:TSZ, :],
                        )
                    # next-to-last chunk's out must not sit on Pool's
                    # sequencer ahead of the trigger; SP is free by then
                    late = USE_TRIG and g == len(CHUNKS) - 2
                    out_eng = nc.sync if (last or late) else nc.gpsimd
                    out_eng.dma_start(
                        outT[:, t0 : t0 + nt, :], out_sb[:TSZ, t0 : t0 + nt, :]
                    )

    if USE_TRIG:
        # The tile sem-assignment schedules the scatter prep on a rotating
        # DMASW lane but the descriptor's completion rides our explicit
        # out5_dma semaphore, so the end-barrier's DMASW wait dangles with
        # no incrementer (deadlock). Drop danglers: program-end ordering is
        # still enforced by the explicit wait_ge(out5_dma) on Pool.
        def _walk(blocks):
            for b in blocks:
                for inst in b.instructions:
                    yield inst
                    if getattr(inst, "blocks", None):
                        yield from _walk(inst.blocks)

        updated = set()
        insts = list(_walk(nc.m.functions[0].blocks))
        for inst in insts:
            si = inst.sync_info
            if si:
                for u in si.on_update:
                    updated.add(u.ant_name)
        for inst in insts:
            si = inst.sync_info
            if si and si.on_wait:
                keep = [
                    w for w in si.on_wait
                    if not (
                        (w.ant_name or "").startswith("DMASW")
                        and w.ant_name not in updated
                    )
                ]
                if len(keep) != len(si.on_wait):
                    si.on_wait = keep

    nc.compile()
    _BUILD_CACHE[key] = nc
    return nc


# ------------------------------------------------------------ input packing
def _pack_shards(query, whl):
    """query [15000, 4096] f32 -> per-core dict of chunk tensors.

    whl [128, KCH, 2, NW_PAD] e3m4 W planes are appended to chunk 0's
    per-k columns so the whole episode needs no separate W DMA.
    """
    f8 = _np_f8()
    xq = (query * np.float32(SX)).astype(f8)          # [15000, 4096] e3m4
    wcols = whl.reshape(128, KCH, 2 * NW_PAD)
    starts = _chunk_starts()
    shards = []
    for c in range(N_CORES):
        qs = xq[c * NQ_SHARD : (c + 1) * NQ_SHARD]    # [1875, 4096]
        chunk_map = {}
        for g, nt in enumerate(CHUNK_TILES):
            csz = nt * TSZ
            q0 = starts[g] * TSZ
            blk = qs[q0 : q0 + csz]                   # [csz, 4096]
            # [csz, KCH, 128] -> [128, KCH, csz]
            arr = blk.reshape(csz, KCH, 128).transpose(2, 1, 0)
            if g == 0:
                arr = np.concatenate([arr, wcols], axis=2)
            chunk_map[f"xt{g}"] = np.ascontiguousarray(arr)
        shards.append(chunk_map)
    return shards


def _pack_w(support, qp):
    """W = sup.T @ qp [4096, 5] -> e3m4 hi/lo planes [128, KCH, 2, NW_PAD].

    Each class column gets its own power-of-2 scale pushing it to the top of
    e3m4's normal range, so the lo plane's residual (denormal floor) is as
    small as possible relative to the column. Returns (whl, col_scales).
    """
    f8 = _np_f8()
    W = np.zeros((D, NW_PAD), np.float32)
    W[:, :N_WAY] = support.T @ qp
    absmax = np.abs(W).max(axis=0)
    scales = np.where(
        absmax > 0,
        np.exp2(np.floor(np.log2(SW_TARGET / np.maximum(absmax, 1e-30)))),
        1.0,
    ).astype(np.float32)
    Wt = W * scales[None, :]
    whi = Wt.astype(f8)
    wlo = (Wt - whi.astype(np.float32)).astype(f8)
    whl = np.zeros((128, KCH, 2, NW_PAD), f8)
    for pl, w in enumerate((whi, wlo)):
        whl[:, :, pl, :] = w.reshape(KCH, 128, NW_PAD).transpose(1, 0, 2)
    return np.ascontiguousarray(whl), scales


def kernel(x):
    x = np.ascontiguousarray(np.asarray(x, dtype=np.float32))
    xr = x.reshape(N_WAY, N_SUPPORT + N_QUERY, D)
    support = np.ascontiguousarray(xr[:, :N_SUPPORT].reshape(NS, D))
    query = np.ascontiguousarray(xr[:, N_SUPPORT:].reshape(NQ_TOT, D))

    # --- host: tiny QP solve (replicated, mirrors reference numerics)
    K = support @ support.T
    qp = _qp_solve_host(K)                              # [25, 5] f32

    whl, col_scales = _pack_w(support, qp)
    shards = _pack_shards(query, whl)

    in_maps = [dict(shards[c]) for c in range(N_CORES)]

    res = None
    last_err = None
    for attempt in range(3):
        try:
            from concourse.bass_utils import run_bass_kernel_spmd

            nc = _build_bass()
            res = run_bass_kernel_spmd(
                nc, in_maps, core_ids=list(range(N_CORES))
            )
            break
        except Exception as e:  # transient device/compile hiccups
            last_err = e
            import sys, time, traceback

            traceback.print_exc()
            word = "retrying" if attempt < 2 else "giving up"
            print(
                f"kernel: device attempt {attempt} failed "
                f"({type(e).__name__}), {word}",
                file=sys.stderr,
            )
            time.sleep(2.0 * (attempt + 1))

    inv = (1.0 / (SX * col_scales[:N_WAY])).astype(np.float32)
    if res is not None:
        logits = np.empty((NQ_TOT, N_WAY), np.float32)
        for c in range(N_CORES):
            outT = np.array(res.results[c]["outT"])     # [125, 15, 8]
            if USE_TRIG:
                outT[:, NT_TOT - 1, :] = res.results[c]["outS"][:TSZ, :NW_PAD]
            logits[c * NQ_SHARD : (c + 1) * NQ_SHARD] = (
                outT.transpose(1, 0, 2).reshape(NQ_SHARD, NW_PAD)[:, :N_WAY]
                * inv[None, :]
            )
        return logits

    # last-resort host fallback: numerically correct, no device speedup
    import sys

    print(
        f"kernel: falling back to host compute after device failure: "
        f"{last_err!r}",
        file=sys.stderr,
    )
    return ((query @ support.T) @ qp).astype(np.float32)


# revision 38
# speedup vs baseline: 1.0250x; 1.0244x over previous
"""MetaOptNet episode kernel for 8x Trainium2 NeuronCores.

Math (from the reference nn.Module):
    x: [15025, 4096] = 5 classes x (5 support + 3000 query) rows.
    K = support @ support.T  (25x25)
    qp = interior-point solve of a tiny 125-var SVM dual (15 fixed iterations)
    logits = (query @ support.T) @ qp        -> [15000, 5]

Split of work:
  - The QP solve is a tiny serial 125-variable problem; it is replicated on
    the host in float32, exactly mirroring the reference algorithm.
  - The memory-bound bulk (streaming the 245 MB of query rows against
    W = support.T @ qp) runs on the 8 NeuronCores, data-parallel over query
    rows (1875 queries per core).

Device kernel design (per core):
  - The query stream is quantized host-side to fp8 e3m4 (x * 2, exactly
    invertible scale), quartering HBM traffic vs the fp32 baseline. W is
    carried as an e3m4 hi+lo pair with per-class power-of-2 scales pushing
    each column to the top of e3m4's normal range, making its quantization
    error negligible; the host divides each logit column by its scale
    afterwards. End-to-end relative error ~1.34e-2 (gate: 2e-2), dominated
    by the x quantization, deterministic on the fixed episode inputs.
  - Matmuls run x-stationary: lhsT = a [128, 125] feature-major query tile
    (PE stationary array, whose load the PE pipelines behind the moving
    pass), rhs = the [128, 8] W chunk (moving, out free size 8) — so the
    tensor engine is far off the critical path. Each query tile accumulates
    its 32 k-chunks x 2 W planes into a private PSUM bank ([125, 8] f32);
    accumulation groups must not share a PSUM bank (bank-granular
    accumulate), so tiles cycle through 6 banks.
  - Queries stream chunk-major (chunks of 3/3/3/3/2/1 tiles of 125): a
    chunk's k-slabs arrive as [128, kslab, csz] fp8 DMAs (>=2KB/partition
    row, full 360 GB/s); outputs are copied and stored while later chunks
    stream, so only the tiny last chunk (+ its 12 post-stream matmuls)
    sits in the serial tail.
  - DMA instruction budget is tuned to the 8 HWDGE + 8 SWDGE completion
    semaphores: 7 stream DMAs + the last out on SP/Activation HWDGE (W
    rides inside chunk 0's stream tensor), the other outs on the Pool
    SWDGE queue — semaphore-slot reuse would chain unrelated DMAs.
"""

import os

import numpy as np

# ---------------------------------------------------------------- constants
N_WAY = 5
N_SUPPORT = 5
N_QUERY = 3000
D = 4096
C_REG = 0.1
MAX_ITER = 15
SIGMA = 0.1

N_CORES = 8
NS = N_WAY * N_SUPPORT          # 25 support rows
NQ_TOT = N_WAY * N_QUERY        # 15000 query rows
NQ_SHARD = NQ_TOT // N_CORES    # 1875 per core
KCH = D // 128                  # 32 contraction chunks of 128
TSZ = 125                       # query tile rows (PSUM group partition dim)
NT_TOT = NQ_SHARD // TSZ        # 15 tiles per core
NW_PAD = int(os.environ.get("MK_NW", "8"))  # class column padding

SX = 2.0                        # x quantization scale (power of 2, exact)
SW_TARGET = 15.0                # per-column W scale target absmax (e3m4 max 15.5)

# chunk layout: "tiles:kslab+kslab+...," per chunk (tiles sum to NT_TOT);
# small chunk last (with a tiny final k-slab) so the serial tail after the
# stream is minimal
_CHUNK_SPEC = os.environ.get(
    "MK_CHUNKS", "3:32,3:32,3:32,3:32,2:32,1:26+6"
)
CHUNKS = tuple(
    (int(part.split(":")[0]),
     tuple(int(s) for s in part.split(":")[1].split("+")))
    for part in _CHUNK_SPEC.split(",")
)
CHUNK_TILES = tuple(nt for nt, _ in CHUNKS)
N_BANKS = int(os.environ.get("MK_BANKS", "6"))  # PSUM banks cycled by tiles
# experimental: last chunk's logits via a pre-generated SWDGE scatter
# descriptor fired by a trigger, skipping the HWDGE+DGE setup (~0.7us) in
# the tail. Default OFF: the prep/trigger path miscompiles in neuronxcc
# (setupSyncUpdate codegen crash) and its completion semaphore bookkeeping
# deadlocks the TimelineSim end barrier without manual sync surgery.
USE_TRIG = os.environ.get("MK_TRIG", "0") == "1"
assert sum(CHUNK_TILES) == NT_TOT
assert all(sum(ks) == KCH for _, ks in CHUNKS)
assert not USE_TRIG or CHUNK_TILES[-1] == 1


def _chunk_starts():
    starts, t0 = [], 0
    for nt in CHUNK_TILES:
        starts.append(t0)
        t0 += nt
    return starts


# ------------------------------------------------------------ host QP solve
def _qp_solve_host(K):
    """Mirror of reference._qp_solve for this problem's fixed G/e/C/h/A/b.

    C is the identity and b is zero, so C-products are elided (exact in
    fp32).  All arithmetic in float32 to track the reference's rounding.
    """
    dt = np.float32
    n = NS * N_WAY                                    # 125
    m, p = n, NS                                      # 125, 25
    G = np.kron(K, np.eye(N_WAY, dtype=dt)).astype(dt) + np.eye(n, dtype=dt)
    y = np.repeat(np.arange(N_WAY), N_SUPPORT)
    y1 = np.eye(N_WAY, dtype=dt)[y].reshape(-1)       # [125] one-hot flat
    e = -y1
    h = (dt(C_REG) * y1).astype(dt)
    A = np.kron(np.eye(NS, dtype=dt), np.ones((1, N_WAY), dtype=dt)).astype(dt)
    sigma = dt(SIGMA)

    z = np.zeros(n, dt)
    s = np.ones(m, dt)
    lam = np.ones(m, dt)
    nu = np.zeros(p, dt)

    for _ in range(MAX_ITER):
        r_dual = G @ z + e + lam + A.T @ nu
        r_pin = z + s - h
        r_peq = A @ z
        mu = np.dot(s, lam) / dt(m)
        r_cent = s * lam - sigma * mu
        w = lam / s
        M = G + np.diag(w).astype(dt)
        rhs_z = -(r_dual + (-r_cent + lam * r_pin) / s)
        KKT = np.block([[M, A.T], [A, np.zeros((p, p), dt)]]).astype(dt)
        sol = np.linalg.solve(KKT, np.concatenate([rhs_z, -r_peq]))
        dz, dnu = sol[:n], sol[n:]
        ds = -r_pin - dz
        dlam = (-r_cent - lam * ds) / s
        with np.errstate(divide="ignore", invalid="ignore"):
            a_s = np.min(np.where(ds < 0, -s / ds, np.inf)).astype(dt)
            a_l = np.min(np.where(dlam < 0, -lam / dlam, np.inf)).astype(dt)
        alpha = np.minimum(dt(1.0), dt(0.99) * np.minimum(a_s, a_l))
        z = z + alpha * dz
        s = s + alpha * ds
        lam = lam + alpha * dlam
        nu = nu + alpha * dnu

    return z.reshape(NS, N_WAY)                       # [25, 5]


# ------------------------------------------------------------- bass builder
_BUILD_CACHE = {}


def _np_f8():
    import ml_dtypes

    return np.dtype(ml_dtypes.float8_e3m4)


def _build_bass():
    key = (CHUNKS, N_BANKS, USE_TRIG)
    if key in _BUILD_CACHE:
        return _BUILD_CACHE[key]

    import concourse.bacc as bacc
    import concourse.mybir as mybir
    import concourse.tile as tile

    f8 = mybir.dt.float8e3
    f32 = mybir.dt.float32
    i16 = mybir.dt.int16

    nc = bacc.Bacc(
        "TRN2", target_bir_lowering=False, debug=False, num_swdge_queues=2
    )
    # chunk 0 carries W appended per k-chunk: [csz stream | 2*NW_PAD W bytes]
    # so no separate W DMA is needed (8-HWDGE-semaphore budget: reusing a
    # semaphore makes a later DMA wait on an unrelated earlier one)
    xts = [
        nc.dram_tensor(
            f"xt{g}",
            [128, KCH, nt * TSZ + (2 * NW_PAD if g == 0 else 0)],
            f8,
            kind="ExternalInput",
        )
        for g, nt in enumerate(CHUNK_TILES)
    ]
    outT = nc.dram_tensor("outT", [TSZ, NT_TOT, NW_PAD], f32, kind="ExternalOutput")
    if USE_TRIG:
        # last tile's rows, one 256B-strided slot per query row (scatter-add
        # destination stride must be a multiple of 256 bytes)
        outS = nc.dram_tensor("outS", [128, 64], f32, kind="ExternalOutput")

    starts = _chunk_starts()

    with tile.TileContext(nc) as tc:
        with (
            tc.tile_pool(name="const", bufs=1) as cpool,
            tc.tile_pool(name="stream", bufs=1) as spool,
            tc.tile_pool(name="acc", bufs=1, space="PSUM") as apool,
        ):
            slabs = {}
            for g, (nt, kslabs) in enumerate(CHUNKS):
                csz = nt * TSZ + (2 * NW_PAD if g == 0 else 0)
                k0 = 0
                for ks in kslabs:
                    slab = spool.tile(
                        [128, ks, csz], f8,
                        tag=f"slab{g}_{k0}", name=f"slab{g}_{k0}",
                    )
                    nc.sync.dma_start(slab[:], xts[g][:, k0 : k0 + ks, :])
                    slabs[g, k0] = (slab, ks)
                    k0 += ks

            if USE_TRIG:
                # pre-generate the last tile's scatter-add descriptors while
                # the stream runs; only the trigger sits in the serial tail
                z8 = cpool.tile([128, NW_PAD], f32, tag="z8")
                nc.gpsimd.memset(z8[:], 0.0)
                s3 = cpool.tile([128, 1, NW_PAD], f32, tag="s3")
                nc.gpsimd.memset(s3[:], 0.0)
                idxs = cpool.tile([16, 8], i16, tag="idx")
                nc.gpsimd.iota(
                    idxs[:], pattern=[[16, 8]], base=0, channel_multiplier=1
                )
                # scatter-add needs its destination payload region zeroed
                # (Pool queue: no waits, runs early, keeps HWDGE slots free)
                nc.gpsimd.dma_start(outS[:, 0:NW_PAD], z8[:])
                sem_out5 = nc.alloc_semaphore("out5_dma")
                sem_s3 = nc.alloc_semaphore("s3_ready")
                nc.gpsimd.dma_scatter_add(
                    outS[:, 0:NW_PAD],
                    s3[:],
                    idxs[:],
                    128,
                    128,
                    NW_PAD,
                    elem_step=64,
                    prepare_only=True,
                    sem=sem_out5,
                    queue_num=1,
                )
            # W slices live inside chunk0's slabs: w_at(k) -> [128, 2*NW_PAD]
            c0 = CHUNK_TILES[0] * TSZ
            k0s_0 = []
            k0 = 0
            for ks in CHUNKS[0][1]:
                k0s_0.append((k0, ks))
                k0 += ks

            def w_at(k, pl):
                for kk0, ks in k0s_0:
                    if kk0 <= k < kk0 + ks:
                        return slabs[0, kk0][0][
                            :, k - kk0, c0 + pl * NW_PAD : c0 + (pl + 1) * NW_PAD
                        ]
                raise AssertionError(k)

            # one PSUM bank per in-flight query tile; tile i -> bank i % N_BANKS
            accs = [
                apool.tile([128, NW_PAD], f32, tag=f"acc{s}", name=f"acc{s}")
                for s in range(N_BANKS)
            ]
            out_sb = cpool.tile([128, NT_TOT, NW_PAD], f32, tag="out")

            for g, (nt, kslabs) in enumerate(CHUNKS):
                t0 = starts[g]
                k0 = 0
                for ks in kslabs:
                    slab, _ = slabs[g, k0]
                    for tl in range(nt):
                        acc = accs[(t0 + tl) % N_BANKS]
                        for kk in range(ks):
                            k = k0 + kk
                            for pl in range(2):
                                nc.tensor.matmul(
                                    acc[:TSZ, :],
                                    slab[:, kk, tl * TSZ : (tl + 1) * TSZ],
                                    w_at(k, pl),
                                    start=(k == 0 and pl == 0),
                                    stop=(k == KCH - 1 and pl == 1),
                                )
                    k0 += ks
                # chunk done: drain its PSUM banks and store its logits.
                # Early outs ride the Pool SWDGE queue (own semaphore space,
                # desc-gen on the otherwise idle Pool engine) so the stream's
                # HWDGE semaphores are never entangled with out completions;
                # the final out fires the pre-generated scatter descriptors
                # (or, without MK_TRIG, a plain SP DMA).
                last = g == len(CHUNKS) - 1
                if last and USE_TRIG:
                    # explicit copy->trigger semaphore: the prep's
                    # deferred-RAW machinery only covers writers that precede
                    # the prep; this copy comes later and would otherwise
                    # race the trigger on real hardware
                    nc.vector.tensor_copy(
                        s3[:TSZ, 0, :], accs[(t0 + 0) % N_BANKS][:TSZ, :]
                    ).then_inc(sem_s3, 1)
                    nc.gpsimd.wait_ge(sem_s3, 1)
                    nc.gpsimd.trigger_dma(count=None, queue_num=1)
                    # completion wait on SP: Pool's sequencer must stay free
                    # for the trigger's descriptor-replay track to run
                    nc.sync.wait_ge(sem_out5, 16)
                else:
                    for tl in range(nt):
                        nc.vector.tensor_copy(
                            out_sb[:TSZ, t0 + tl, :],
                            accs[(t0 + tl) % N_BANKS][:TSZ, :],
                        )
                    # with the trigger gated on the copy semaphore, chunk 4's
                    # Pool desc-gen overlaps the last chunk's copy chain, so
                    # all non-final outs ride Pool; without the trigger the
                    # final out itself needs the SP HWDGE slot
                    out_eng = nc.sync if last else nc.gpsimd
                    out_eng.dma_start(
                        outT[:, t0 : t0 + nt, :], out_sb[:TSZ, t0 : t0 + nt, :]
                    )

    if USE_TRIG:
        # The tile sem-assignment schedules the scatter prep on a rotating
        # DMASW lane but the descriptor's completion rides our explicit
        # out5_dma semaphore, so the end-barrier's DMASW wait dangles with
        # no incrementer (deadlock). Drop danglers: program-end ordering is
        # still enforced by the explicit wait_ge(out5_dma) on Pool.
        def _walk(blocks):
            for b in blocks:
                for inst in b.instructions:
                    yield inst
                    if getattr(inst, "blocks", None):
                        yield from _walk(inst.blocks)

        updated = set()
        insts = list(_walk(nc.m.functions[0].blocks))
        for inst in insts:
            si = inst.sync_info
            if si:
                for u in si.on_update:
                    updated.add(u.ant_name)
        for inst in insts:
            si = inst.sync_info
            if si and si.on_wait:
                keep = [
                    w for w in si.on_wait
                    if not (
                        (w.ant_name or "").startswith("DMASW")
                        and w.ant_name not in updated
                    )
                ]
                if len(keep) != len(si.on_wait):
                    si.on_wait = keep

    nc.compile()
    _BUILD_CACHE[key] = nc
    return nc


# ------------------------------------------------------------ input packing
def _pack_shards(query, whl):
    """query [15000, 4096] f32 -> per-core dict of chunk tensors.

    whl [128, KCH, 2, NW_PAD] e3m4 W planes are appended to chunk 0's
    per-k columns so the whole episode needs no separate W DMA.
    """
    f8 = _np_f8()
    xq = (query * np.float32(SX)).astype(f8)          # [15000, 4096] e3m4
    wcols = whl.reshape(128, KCH, 2 * NW_PAD)
    starts = _chunk_starts()
    shards = []
    for c in range(N_CORES):
        qs = xq[c * NQ_SHARD : (c + 1) * NQ_SHARD]    # [1875, 4096]
        chunk_map = {}
        for g, nt in enumerate(CHUNK_TILES):
            csz = nt * TSZ
            q0 = starts[g] * TSZ
            blk = qs[q0 : q0 + csz]                   # [csz, 4096]
            # [csz, KCH, 128] -> [128, KCH, csz]
            arr = blk.reshape(csz, KCH, 128).transpose(2, 1, 0)
            if g == 0:
                arr = np.concatenate([arr, wcols], axis=2)
            chunk_map[f"xt{g}"] = np.ascontiguousarray(arr)
        shards.append(chunk_map)
    return shards


def _pack_w(support, qp):
    """W = sup.T @ qp [4096, 5] -> e3m4 hi/lo planes [128, KCH, 2, NW_PAD].

    Each class column gets its own power-of-2 scale pushing it to the top of
    e3m4's normal range, so the lo plane's residual (denormal floor) is as
    small as possible relative to the column. Returns (whl, col_scales).
    """
    f8 = _np_f8()
    W = np.zeros((D, NW_PAD), np.float32)
    W[:, :N_WAY] = support.T @ qp
    absmax = np.abs(W).max(axis=0)
    scales = np.where(
        absmax > 0,
        np.exp2(np.floor(np.log2(SW_TARGET / np.maximum(absmax, 1e-30)))),
        1.0,
    ).astype(np.float32)
    Wt = W * scales[None, :]
    whi = Wt.astype(f8)
    wlo = (Wt - whi.astype(np.float32)).astype(f8)
    whl = np.zeros((128, KCH, 2, NW_PAD), f8)
    for pl, w in enumerate((whi, wlo)):
        whl[:, :, pl, :] = w.reshape(KCH, 128, NW_PAD).transpose(1, 0, 2)
    return np.ascontiguousarray(whl), scales


def kernel(x):
    x = np.ascontiguousarray(np.asarray(x, dtype=np.float32))
    xr = x.reshape(N_WAY, N_SUPPORT + N_QUERY, D)
    support = np.ascontiguousarray(xr[:, :N_SUPPORT].reshape(NS, D))
    query = np.ascontiguousarray(xr[:, N_SUPPORT:].reshape(NQ_TOT, D))

    # --- host: tiny QP solve (replicated, mirrors reference numerics)
    K = support @ support.T
    qp = _qp_solve_host(K)                              # [25, 5] f32

    whl, col_scales = _pack_w(support, qp)
    shards = _pack_shards(query, whl)

    in_maps = [dict(shards[c]) for c in range(N_CORES)]

    res = None
    last_err = None
    for attempt in range(3):
        try:
            from concourse.bass_utils import run_bass_kernel_spmd

            nc = _build_bass()
            res = run_bass_kernel_spmd(
                nc, in_maps, core_ids=list(range(N_CORES))
            )
            break
        except Exception as e:  # transient device/compile hiccups
            last_err = e
            import sys, time, traceback

            traceback.print_exc()
            word = "retrying" if attempt < 2 else "giving up"
            print(
                f"kernel: device attempt {attempt} failed "
                f"({type(e).__name__}), {word}",
                file=sys.stderr,
            )
            time.sleep(2.0 * (attempt + 1))

    inv = (1.0 / (SX * col_scales[:N_WAY])).astype(np.float32)
    if res is not None:
        logits = np.empty((NQ_TOT, N_WAY), np.float32)
        for c in range(N_CORES):
            outT = np.array(res.results[c]["outT"])     # [125, 15, 8]
            if USE_TRIG:
                outT[:, NT_TOT - 1, :] = res.results[c]["outS"][:TSZ, :NW_PAD]
            logits[c * NQ_SHARD : (c + 1) * NQ_SHARD] = (
                outT.transpose(1, 0, 2).reshape(NQ_SHARD, NW_PAD)[:, :N_WAY]
                * inv[None, :]
            )
        return logits

    # last-resort host fallback: numerically correct, no device speedup
    import sys

    print(
        f"kernel: falling back to host compute after device failure: "
        f"{last_err!r}",
        file=sys.stderr,
    )
    return ((query @ support.T) @ qp).astype(np.float32)


# revision 41
# speedup vs baseline: 1.0273x; 1.0023x over previous
"""MetaOptNet episode kernel for 8x Trainium2 NeuronCores.

Math (from the reference nn.Module):
    x: [15025, 4096] = 5 classes x (5 support + 3000 query) rows.
    K = support @ support.T  (25x25)
    qp = interior-point solve of a tiny 125-var SVM dual (15 fixed iterations)
    logits = (query @ support.T) @ qp        -> [15000, 5]

Split of work:
  - The QP solve is a tiny serial 125-variable problem; it is replicated on
    the host in float32, exactly mirroring the reference algorithm.
  - The memory-bound bulk (streaming the 245 MB of query rows against
    W = support.T @ qp) runs on the 8 NeuronCores, data-parallel over query
    rows (1875 queries per core).

Device kernel design (per core):
  - The query stream is quantized host-side to fp8 e3m4 (x * 2, exactly
    invertible scale), quartering HBM traffic vs the fp32 baseline. W is
    carried as an e3m4 hi+lo pair with per-class power-of-2 scales pushing
    each column to the top of e3m4's normal range, making its quantization
    error negligible; the host divides each logit column by its scale
    afterwards. End-to-end relative error ~1.34e-2 (gate: 2e-2), dominated
    by the x quantization, deterministic on the fixed episode inputs.
  - Matmuls run x-stationary: lhsT = a [128, 125] feature-major query tile
    (PE stationary array, whose load the PE pipelines behind the moving
    pass), rhs = the [128, 8] W chunk (moving, out free size 8) — so the
    tensor engine is far off the critical path. Each query tile accumulates
    its 32 k-chunks x 2 W planes into a private PSUM bank ([125, 8] f32);
    accumulation groups must not share a PSUM bank (bank-granular
    accumulate), so tiles cycle through 6 banks.
  - Queries stream chunk-major (chunks of 3/3/3/3/2/1 tiles of 125): a
    chunk's k-slabs arrive as [128, kslab, csz] fp8 DMAs (>=2KB/partition
    row, full 360 GB/s); outputs are copied and stored while later chunks
    stream, so only the tiny last chunk (+ its 12 post-stream matmuls)
    sits in the serial tail.
  - DMA instruction budget is tuned to the 8 HWDGE + 8 SWDGE completion
    semaphores: 7 stream DMAs + the last out on SP/Activation HWDGE (W
    rides inside chunk 0's stream tensor), the other outs on the Pool
    SWDGE queue — semaphore-slot reuse would chain unrelated DMAs.
"""

import os

import numpy as np

# ---------------------------------------------------------------- constants
N_WAY = 5
N_SUPPORT = 5
N_QUERY = 3000
D = 4096
C_REG = 0.1
MAX_ITER = 15
SIGMA = 0.1

N_CORES = 8
NS = N_WAY * N_SUPPORT          # 25 support rows
NQ_TOT = N_WAY * N_QUERY        # 15000 query rows
NQ_SHARD = NQ_TOT // N_CORES    # 1875 per core
KCH = D // 128                  # 32 contraction chunks of 128
TSZ = 125                       # query tile rows (PSUM group partition dim)
NT_TOT = NQ_SHARD // TSZ        # 15 tiles per core
NW_PAD = int(os.environ.get("MK_NW", "8"))  # class column padding

SX = 2.0                        # x quantization scale (power of 2, exact)
SW_TARGET = 15.0                # per-column W scale target absmax (e3m4 max 15.5)

# chunk layout: "tiles:kslab+kslab+...," per chunk (tiles sum to NT_TOT);
# small chunk last (with a tiny final k-slab) so the serial tail after the
# stream is minimal
_CHUNK_SPEC = os.environ.get(
    "MK_CHUNKS", "3:32,3:32,3:32,3:32,2:32,1:26+6"
)
CHUNKS = tuple(
    (int(part.split(":")[0]),
     tuple(int(s) for s in part.split(":")[1].split("+")))
    for part in _CHUNK_SPEC.split(",")
)
CHUNK_TILES = tuple(nt for nt, _ in CHUNKS)
N_BANKS = int(os.environ.get("MK_BANKS", "6"))  # PSUM banks cycled by tiles
# experimental: last chunk's logits via a pre-generated SWDGE scatter
# descriptor fired by a trigger, skipping the HWDGE+DGE setup (~0.7us) in
# the tail. Default OFF: the prep/trigger path miscompiles in neuronxcc
# (setupSyncUpdate codegen crash) and its completion semaphore bookkeeping
# deadlocks the TimelineSim end barrier without manual sync surgery.
USE_TRIG = os.environ.get("MK_TRIG", "0") == "1"
assert sum(CHUNK_TILES) == NT_TOT
assert all(sum(ks) == KCH for _, ks in CHUNKS)
assert not USE_TRIG or CHUNK_TILES[-1] == 1


def _chunk_starts():
    starts, t0 = [], 0
    for nt in CHUNK_TILES:
        starts.append(t0)
        t0 += nt
    return starts


# ------------------------------------------------------------ host QP solve
def _qp_solve_host(K):
    """Mirror of reference._qp_solve for this problem's fixed G/e/C/h/A/b.

    C is the identity and b is zero, so C-products are elided (exact in
    fp32).  All arithmetic in float32 to track the reference's rounding.
    """
    dt = np.float32
    n = NS * N_WAY                                    # 125
    m, p = n, NS                                      # 125, 25
    G = np.kron(K, np.eye(N_WAY, dtype=dt)).astype(dt) + np.eye(n, dtype=dt)
    y = np.repeat(np.arange(N_WAY), N_SUPPORT)
    y1 = np.eye(N_WAY, dtype=dt)[y].reshape(-1)       # [125] one-hot flat
    e = -y1
    h = (dt(C_REG) * y1).astype(dt)
    A = np.kron(np.eye(NS, dtype=dt), np.ones((1, N_WAY), dtype=dt)).astype(dt)
    sigma = dt(SIGMA)

    z = np.zeros(n, dt)
    s = np.ones(m, dt)
    lam = np.ones(m, dt)
    nu = np.zeros(p, dt)

    for _ in range(MAX_ITER):
        r_dual = G @ z + e + lam + A.T @ nu
        r_pin = z + s - h
        r_peq = A @ z
        mu = np.dot(s, lam) / dt(m)
        r_cent = s * lam - sigma * mu
        w = lam / s
        M = G + np.diag(w).astype(dt)
        rhs_z = -(r_dual + (-r_cent + lam * r_pin) / s)
        KKT = np.block([[M, A.T], [A, np.zeros((p, p), dt)]]).astype(dt)
        sol = np.linalg.solve(KKT, np.concatenate([rhs_z, -r_peq]))
        dz, dnu = sol[:n], sol[n:]
        ds = -r_pin - dz
        dlam = (-r_cent - lam * ds) / s
        with np.errstate(divide="ignore", invalid="ignore"):
            a_s = np.min(np.where(ds < 0, -s / ds, np.inf)).astype(dt)
            a_l = np.min(np.where(dlam < 0, -lam / dlam, np.inf)).astype(dt)
        alpha = np.minimum(dt(1.0), dt(0.99) * np.minimum(a_s, a_l))
        z = z + alpha * dz
        s = s + alpha * ds
        lam = lam + alpha * dlam
        nu = nu + alpha * dnu

    return z.reshape(NS, N_WAY)                       # [25, 5]


# ------------------------------------------------------------- bass builder
_BUILD_CACHE = {}


def _np_f8():
    import ml_dtypes

    return np.dtype(ml_dtypes.float8_e3m4)


def _build_bass():
    key = (CHUNKS, N_BANKS, USE_TRIG)
    if key in _BUILD_CACHE:
        return _BUILD_CACHE[key]

    import concourse.bacc as bacc
    import concourse.mybir as mybir
    import concourse.tile as tile

    f8 = mybir.dt.float8e3
    f32 = mybir.dt.float32
    i16 = mybir.dt.int16

    nc = bacc.Bacc(
        "TRN2", target_bir_lowering=False, debug=False, num_swdge_queues=2
    )
    # chunk 0 carries W appended per k-chunk: [csz stream | 2*NW_PAD W bytes]
    # so no separate W DMA is needed (8-HWDGE-semaphore budget: reusing a
    # semaphore makes a later DMA wait on an unrelated earlier one)
    xts = [
        nc.dram_tensor(
            f"xt{g}",
            [128, KCH, nt * TSZ + (2 * NW_PAD if g == 0 else 0)],
            f8,
            kind="ExternalInput",
        )
        for g, nt in enumerate(CHUNK_TILES)
    ]
    outT = nc.dram_tensor("outT", [TSZ, NT_TOT, NW_PAD], f32, kind="ExternalOutput")
    if USE_TRIG:
        # last tile's rows, one 256B-strided slot per query row (scatter-add
        # destination stride must be a multiple of 256 bytes)
        outS = nc.dram_tensor("outS", [128, 64], f32, kind="ExternalOutput")

    starts = _chunk_starts()

    with tile.TileContext(nc) as tc:
        with (
            tc.tile_pool(name="const", bufs=1) as cpool,
            tc.tile_pool(name="stream", bufs=1) as spool,
            tc.tile_pool(name="acc", bufs=1, space="PSUM") as apool,
        ):
            slabs = {}
            for g, (nt, kslabs) in enumerate(CHUNKS):
                csz = nt * TSZ + (2 * NW_PAD if g == 0 else 0)
                k0 = 0
                for ks in kslabs:
                    slab = spool.tile(
                        [128, ks, csz], f8,
                        tag=f"slab{g}_{k0}", name=f"slab{g}_{k0}",
                    )
                    nc.sync.dma_start(slab[:], xts[g][:, k0 : k0 + ks, :])
                    slabs[g, k0] = (slab, ks)
                    k0 += ks

            if USE_TRIG:
                # pre-generate the last tile's scatter-add descriptors while
                # the stream runs; only the trigger sits in the serial tail
                z8 = cpool.tile([128, NW_PAD], f32, tag="z8")
                nc.gpsimd.memset(z8[:], 0.0)
                s3 = cpool.tile([128, 1, NW_PAD], f32, tag="s3")
                nc.gpsimd.memset(s3[:], 0.0)
                idxs = cpool.tile([16, 8], i16, tag="idx")
                nc.gpsimd.iota(
                    idxs[:], pattern=[[16, 8]], base=0, channel_multiplier=1
                )
                # scatter-add needs its destination payload region zeroed
                # (Pool queue: no waits, runs early, keeps HWDGE slots free)
                nc.gpsimd.dma_start(outS[:, 0:NW_PAD], z8[:])
                sem_out5 = nc.alloc_semaphore("out5_dma")
                pscr = cpool.tile([128, 1], f32, tag="pscr")
                nc.gpsimd.dma_scatter_add(
                    outS[:, 0:NW_PAD],
                    s3[:],
                    idxs[:],
                    128,
                    128,
                    NW_PAD,
                    elem_step=64,
                    prepare_only=True,
                    sem=sem_out5,
                    queue_num=1,
                )
            # W slices live inside chunk0's slabs: w_at(k) -> [128, 2*NW_PAD]
            c0 = CHUNK_TILES[0] * TSZ
            k0s_0 = []
            k0 = 0
            for ks in CHUNKS[0][1]:
                k0s_0.append((k0, ks))
                k0 += ks

            def w_at(k, pl):
                for kk0, ks in k0s_0:
                    if kk0 <= k < kk0 + ks:
                        return slabs[0, kk0][0][
                            :, k - kk0, c0 + pl * NW_PAD : c0 + (pl + 1) * NW_PAD
                        ]
                raise AssertionError(k)

            # one PSUM bank per in-flight query tile; tile i -> bank i % N_BANKS
            accs = [
                apool.tile([128, NW_PAD], f32, tag=f"acc{s}", name=f"acc{s}")
                for s in range(N_BANKS)
            ]
            out_sb = cpool.tile([128, NT_TOT, NW_PAD], f32, tag="out")

            for g, (nt, kslabs) in enumerate(CHUNKS):
                t0 = starts[g]
                k0 = 0
                for ks in kslabs:
                    slab, _ = slabs[g, k0]
                    for tl in range(nt):
                        acc = accs[(t0 + tl) % N_BANKS]
                        for kk in range(ks):
                            k = k0 + kk
                            for pl in range(2):
                                nc.tensor.matmul(
                                    acc[:TSZ, :],
                                    slab[:, kk, tl * TSZ : (tl + 1) * TSZ],
                                    w_at(k, pl),
                                    start=(k == 0 and pl == 0),
                                    stop=(k == KCH - 1 and pl == 1),
                                )
                    k0 += ks
                # chunk done: drain its PSUM banks and store its logits.
                # Early outs ride the Pool SWDGE queue (own semaphore space,
                # desc-gen on the otherwise idle Pool engine) so the stream's
                # HWDGE semaphores are never entangled with out completions;
                # the final out fires the pre-generated scatter descriptors
                # (or, without MK_TRIG, a plain SP DMA).
                last = g == len(CHUNKS) - 1
                if last and USE_TRIG:
                    nc.vector.tensor_copy(
                        s3[:TSZ, 0, :], accs[(t0 + 0) % N_BANKS][:TSZ, :]
                    )
                    # tiny Pool-side read of s3 orders the trigger after the
                    # copy via a tile-managed RAW edge (the prep's deferred-
                    # RAW machinery only covers writers preceding the prep,
                    # and extra then_inc updates overflow TensorCopy's ISA
                    # sync-update slots)
                    nc.gpsimd.tensor_copy(pscr[:1, :1], s3[:1, 0, :1])
                    nc.gpsimd.trigger_dma(count=None, queue_num=1)
                    # completion wait on SP: Pool's sequencer must stay free
                    # for the trigger's descriptor-replay track to run
                    nc.sync.wait_ge(sem_out5, 16)
                else:
                    for tl in range(nt):
                        nc.vector.tensor_copy(
                            out_sb[:TSZ, t0 + tl, :],
                            accs[(t0 + tl) % N_BANKS][:TSZ, :],
                        )
                    # chunk 4's out must stay off Pool: the scheduler sinks
                    # the scatter prep adjacent to its trigger, so any heavy
                    # Pool desc-gen emitted before the trigger lands on the
                    # tail's critical chain
                    late = USE_TRIG and g == len(CHUNKS) - 2
                    out_eng = nc.sync if (last or late) else nc.gpsimd
                    out_eng.dma_start(
                        outT[:, t0 : t0 + nt, :], out_sb[:TSZ, t0 : t0 + nt, :]
                    )

    if USE_TRIG:
        # The tile sem-assignment schedules the scatter prep on a rotating
        # DMASW lane but the descriptor's completion rides our explicit
        # out5_dma semaphore, so the end-barrier's DMASW wait dangles with
        # no incrementer (deadlock). Drop danglers: program-end ordering is
        # still enforced by the explicit wait_ge(out5_dma) on Pool.
        def _walk(blocks):
            for b in blocks:
                for inst in b.instructions:
                    yield inst
                    if getattr(inst, "blocks", None):
                        yield from _walk(inst.blocks)

        updated = set()
        insts = list(_walk(nc.m.functions[0].blocks))
        for inst in insts:
            si = inst.sync_info
            if si:
                for u in si.on_update:
                    updated.add(u.ant_name)
        for inst in insts:
            si = inst.sync_info
            if si and si.on_wait:
                keep = [
                    w for w in si.on_wait
                    if not (
                        (w.ant_name or "").startswith("DMASW")
                        and w.ant_name not in updated
                    )
                ]
                if len(keep) != len(si.on_wait):
                    si.on_wait = keep

    nc.compile()
    _BUILD_CACHE[key] = nc
    return nc


# ------------------------------------------------------------ input packing
def _pack_shards(query, whl):
    """query [15000, 4096] f32 -> per-core dict of chunk tensors.

    whl [128, KCH, 2, NW_PAD] e3m4 W planes are appended to chunk 0's
    per-k columns so the whole episode needs no separate W DMA.
    """
    f8 = _np_f8()
    xq = (query * np.float32(SX)).astype(f8)          # [15000, 4096] e3m4
    wcols = whl.reshape(128, KCH, 2 * NW_PAD)
    starts = _chunk_starts()
    shards = []
    for c in range(N_CORES):
        qs = xq[c * NQ_SHARD : (c + 1) * NQ_SHARD]    # [1875, 4096]
        chunk_map = {}
        for g, nt in enumerate(CHUNK_TILES):
            csz = nt * TSZ
            q0 = starts[g] * TSZ
            blk = qs[q0 : q0 + csz]                   # [csz, 4096]
            # [csz, KCH, 128] -> [128, KCH, csz]
            arr = blk.reshape(csz, KCH, 128).transpose(2, 1, 0)
            if g == 0:
                arr = np.concatenate([arr, wcols], axis=2)
            chunk_map[f"xt{g}"] = np.ascontiguousarray(arr)
        shards.append(chunk_map)
    return shards


def _pack_w(support, qp):
    """W = sup.T @ qp [4096, 5] -> e3m4 hi/lo planes [128, KCH, 2, NW_PAD].

    Each class column gets its own power-of-2 scale pushing it to the top of
    e3m4's normal range, so the lo plane's residual (denormal floor) is as
    small as possible relative to the column. Returns (whl, col_scales).
    """
    f8 = _np_f8()
    W = np.zeros((D, NW_PAD), np.float32)
    W[:, :N_WAY] = support.T @ qp
    absmax = np.abs(W).max(axis=0)
    scales = np.where(
        absmax > 0,
        np.exp2(np.floor(np.log2(SW_TARGET / np.maximum(absmax, 1e-30)))),
        1.0,
    ).astype(np.float32)
    Wt = W * scales[None, :]
    whi = Wt.astype(f8)
    wlo = (Wt - whi.astype(np.float32)).astype(f8)
    whl = np.zeros((128, KCH, 2, NW_PAD), f8)
    for pl, w in enumerate((whi, wlo)):
        whl[:, :, pl, :] = w.reshape(KCH, 128, NW_PAD).transpose(1, 0, 2)
    return np.ascontiguousarray(whl), scales


def kernel(x):
    x = np.ascontiguousarray(np.asarray(x, dtype=np.float32))
    xr = x.reshape(N_WAY, N_SUPPORT + N_QUERY, D)
    support = np.ascontiguousarray(xr[:, :N_SUPPORT].reshape(NS, D))
    query = np.ascontiguousarray(xr[:, N_SUPPORT:].reshape(NQ_TOT, D))

    # --- host: tiny QP solve (replicated, mirrors reference numerics)
    K = support @ support.T
    qp = _qp_solve_host(K)                              # [25, 5] f32

    whl, col_scales = _pack_w(support, qp)
    shards = _pack_shards(query, whl)

    in_maps = [dict(shards[c]) for c in range(N_CORES)]

    res = None
    last_err = None
    for attempt in range(3):
        try:
            from concourse.bass_utils import run_bass_kernel_spmd

            nc = _build_bass()
            res = run_bass_kernel_spmd(
                nc, in_maps, core_ids=list(range(N_CORES))
            )
            break
        except Exception as e:  # transient device/compile hiccups
            last_err = e
            import sys, time, traceback

            traceback.print_exc()
            word = "retrying" if attempt < 2 else "giving up"
            print(
                f"kernel: device attempt {attempt} failed "
                f"({type(e).__name__}), {word}",
                file=sys.stderr,
            )
            time.sleep(2.0 * (attempt + 1))

    inv = (1.0 / (SX * col_scales[:N_WAY])).astype(np.float32)
    if res is not None:
        logits = np.empty((NQ_TOT, N_WAY), np.float32)
        for c in range(N_CORES):
            outT = np.array(res.results[c]["outT"])     # [125, 15, 8]
            if USE_TRIG:
                outT[:, NT_TOT - 1, :] = res.results[c]["outS"][:TSZ, :NW_PAD]
            logits[c * NQ_SHARD : (c + 1) * NQ_SHARD] = (
                outT.transpose(1, 0, 2).reshape(NQ_SHARD, NW_PAD)[:, :N_WAY]
                * inv[None, :]
            )
        return logits

    # last-resort host fallback: numerically correct, no device speedup
    import sys

    print(
        f"kernel: falling back to host compute after device failure: "
        f"{last_err!r}",
        file=sys.stderr,
    )
    return ((query @ support.T) @ qp).astype(np.float32)
